# revision 35
# baseline (speedup 1.0000x reference)
"""Trainium2 Bass kernel for nn_GAT_7507602833557 (8-core SPMD GAT).

Sharding: query-node rows split across 8 cores (512 rows each); keys/values
replicated. Per-core adjacency passed pre-transposed ([keys, own queries]) in
bf16 ({0,1} exact).

Math (per attention map): exp(lrelu(f1[i]+f2[j])) = e^{f1}·C[j]·max(P[j],Q[i])
with P=e^{0.8 f2}, C=e^{0.2 f2}, Q=e^{-0.8 f1}; e^{f1} cancels in softmax.
C is folded into the value matrix (whp, incl. its ones column -> denominator),
so the map is one single-op tensor_scalar (max) + one mask multiply.

Layer-1 matmuls run "flipped": stationary = whp [128k, 33], moving = the
masked map m [128k, 512q], accumulating [33, 512] per head across 32 key
tiles into 8 rotating PSUM banks. Result is feature-major, which feeds the
output-layer W_out matmul directly (no transposes); softmax division uses
reciprocal_approx_fast + a PE ones-broadcast.
"""
import numpy as np

import concourse.bass as bass
import concourse.bacc as bacc
import concourse.tile as tile
from concourse import mybir
from concourse.bass_utils import run_bass_kernel_spmd
from concourse.masks import make_identity

import ml_dtypes

N, NIN, NHID, NOUT = 4096, 64, 32, 64
NHEADS, D_INT = 8, 32
H_SP, H_INT = 6, 2
NCORES = 8
R = N // NCORES           # 512 own query rows per core
JT = N // 128             # 32 key tiles
F32 = mybir.dt.float32
BF16 = mybir.dt.bfloat16
POOL_HEADS = 2            # trailing heads whose mask-mult runs on gpsimd
L2_POOL_STRIDE = 4        # every 4th output-layer mask-mult on gpsimd


def _build_program():
    nc = bacc.Bacc("TRN2", target_bir_lowering=False, debug=False,
                   num_devices=NCORES)
    d_xi = nc.dram_tensor("xi", [NIN + D_INT, N], F32, kind="ExternalInput")
    d_xb = nc.dram_tensor("xb", [NIN, N], BF16, kind="ExternalInput")
    d_adjT = nc.dram_tensor("adjT", [128, JT * R], BF16, kind="ExternalInput")
    d_xo = nc.dram_tensor("xoT", [NIN, R], BF16, kind="ExternalInput")
    d_io = nc.dram_tensor("ioT", [D_INT, R], BF16, kind="ExternalInput")
    d_wia = nc.dram_tensor("wia", [NIN + D_INT, 16], F32, kind="ExternalInput")
    d_wqa = nc.dram_tensor("wqa", [NIN + D_INT, 10], BF16, kind="ExternalInput")
    d_qsel = nc.dram_tensor("qsel", [10, NHEADS * 128], BF16, kind="ExternalInput")
    d_wb = nc.dram_tensor("wb", [NIN, NHEADS * NHID], BF16, kind="ExternalInput")
    d_w4 = nc.dram_tensor("w4", [4, 128, NOUT], BF16, kind="ExternalInput")
    d_aout = nc.dram_tensor("aout", [2 * NOUT], F32, kind="ExternalInput")
    d_esel = nc.dram_tensor("esel", [NHEADS, R], F32, kind="ExternalInput")
    d_out = nc.dram_tensor("out", [R, NOUT], F32, kind="ExternalOutput")

    with tile.TileContext(nc) as tc:
        _kernel_body(tc, d_xi, d_xb, d_adjT, d_xo, d_io, d_wia, d_wqa,
                     d_qsel, d_wb, d_w4, d_aout, d_esel, d_out)
    nc.compile()
    return nc


def _kernel_body(tc, d_xi, d_xb, d_adjT, d_xo, d_io, d_wia, d_wqa, d_qsel,
                 d_wb, d_w4, d_aout, d_esel, d_out):
    nc = tc.nc
    Act = mybir.ActivationFunctionType
    Alu = mybir.AluOpType
    from contextlib import ExitStack

    ctx = ExitStack()
    big = ctx.enter_context(tc.tile_pool(name="big", bufs=1))
    work = ctx.enter_context(tc.tile_pool(name="work", bufs=2))
    ext = ctx.enter_context(tc.tile_pool(name="ext", bufs=2))
    dram = ctx.enter_context(tc.tile_pool(name="dram", bufs=1, space="DRAM"))
    try:
        _body(tc, ctx, big, work, ext, dram, d_xi, d_xb, d_adjT, d_xo,
              d_io, d_wia, d_wqa, d_qsel, d_wb, d_w4, d_aout, d_esel, d_out)
    finally:
        ctx.close()


def _body(tc, ctx, big, work, ext, dram, d_xi, d_xb, d_adjT, d_xo, d_io,
          d_wia, d_wqa, d_qsel, d_wb, d_w4, d_aout, d_esel, d_out):
    nc = tc.nc
    Act = mybir.ActivationFunctionType
    Alu = mybir.AluOpType
    from contextlib import ExitStack

    # ---------------- loads (critical-path-first order) --------------------
    wqa = big.tile([NIN + D_INT, 10], BF16, tag="wqa")
    nc.sync.dma_start(out=wqa, in_=d_wqa.ap())
    qsel = big.tile([10, NHEADS * 128], BF16, tag="qsel")
    nc.sync.dma_start(out=qsel, in_=d_qsel.ap())
    xio = big.tile([NIN + D_INT, R], BF16, tag="xio")
    nc.sync.dma_start(out=xio[0:NIN, :], in_=d_xo.ap())
    nc.sync.dma_start(out=xio[NIN:NIN + D_INT, :], in_=d_io.ap())
    wia = big.tile([NIN + D_INT, 16], F32, tag="wia")
    nc.sync.dma_start(out=wia, in_=d_wia.ap())
    xi = big.tile([NIN + D_INT, N], F32, tag="xi")
    nc.sync.dma_start(out=xi[:, 0:1024], in_=d_xi.ap()[:, 0:1024])
    wb = big.tile([NIN, NHEADS * NHID], BF16, tag="wb")
    nc.sync.dma_start(out=wb, in_=d_wb.ap())
    adjT_sb = big.tile([128, JT, R], BF16, tag="adjT_sb")
    nc.sync.dma_start(
        out=adjT_sb[:, 0:4, :],
        in_=d_adjT.ap()[:, 0:4 * R].rearrange("p (t i) -> p t i", i=R))
    xb = big.tile([NIN, N], BF16, tag="xb")
    nc.sync.dma_start(out=xb, in_=d_xb.ap())
    for g in range(1, 4):
        nc.sync.dma_start(
            out=xi[:, 1024 * g:1024 * (g + 1)],
            in_=d_xi.ap()[:, 1024 * g:1024 * (g + 1)])
    for g in range(1, 8):
        nc.sync.dma_start(
            out=adjT_sb[:, 4 * g:4 * (g + 1), :],
            in_=d_adjT.ap()[:, 4 * g * R:4 * (g + 1) * R]
                .rearrange("p (t i) -> p t i", i=R))
    w4sb = big.tile([128, 4, NOUT], BF16, tag="w4sb")
    nc.sync.dma_start(out=w4sb, in_=d_w4.ap().rearrange("c p o -> p c o"))
    esel = big.tile([NHEADS, R], F32, tag="esel")
    nc.sync.dma_start(out=esel, in_=d_esel.ap())
    aout_f = big.tile([NOUT, 2], F32, tag="aout_f")
    nc.sync.dma_start(out=aout_f, in_=d_aout.ap().rearrange("(c o) -> o c", c=2))

    ps_setup = ExitStack()
    psum = ps_setup.enter_context(tc.tile_pool(name="ps_setup", bufs=3,
                                               space="PSUM"))

    dummy_act = big.tile([1, 2], F32, tag="dummy_act")
    nc.vector.memset(dummy_act, 0.5)
    nc.scalar.activation(out=dummy_act, in_=dummy_act, func=Act.Exp)

    # -------- Q/R rows (one stacked matmul) + qb broadcast via selector ----
    pq = psum.tile([10, R], F32, tag="ps")
    nc.tensor.matmul(pq, wqa, xio)
    qrow = big.tile([10, R], BF16, tag="qrow")
    nc.scalar.activation(out=qrow, in_=pq, func=Act.Exp)
    negones = big.tile([128, 1], F32, tag="negones")
    nc.vector.memset(negones, -1.0)
    qb = big.tile([128, NHEADS, R], BF16, tag="qb")
    for h in [4, 5, 6, 7, 0, 1, 2, 3]:
        pqb = psum.tile([128, R], F32, tag="ps")
        nc.tensor.matmul(pqb, qsel[:, 128 * h:128 * (h + 1)], qrow)
        nc.vector.tensor_copy(qb[:, h, :], pqb)

    # ------- layer-1 attention; ET/whp production interleaved per jc -------
    # et cols 0:6 P_sp, 6:12 C_sp, 12:14 P_int, 14:16 C_int
    et = big.tile([128, JT, 16], F32, tag="et")
    et16 = big.tile([128, JT, 16], BF16, tag="et16")
    whp = big.tile([128, JT, NHEADS, NHID + 1], BF16, tag="whp")
    ps_acc = ExitStack()
    pacc = ps_acc.enter_context(tc.tile_pool(name="pacc", bufs=1, space="PSUM"))
    apk = [pacc.tile([64 + NHID + 1, R], F32, tag=f"apk{c}", name=f"apk{c}")
           for c in range(4)]
    accs = [apk[h // 2][64 * (h % 2):64 * (h % 2) + NHID + 1, :]
            for h in range(NHEADS)]
    HORDER = [6, 7, 0, 1, 2, 3, 4, 5]
    wag_in = dram.tile([16, 2], BF16, tag="wag_in")
    wag_out = dram.tile([128, 2], BF16, tag="wag_out")
    nc.sync.dma_start(out=wag_in, in_=d_adjT.ap()[0:16, 0:2])
    for jc in range(JT):
        # production for this key tile (runs ahead on tensor/scalar/gpsimd)
        pf = psum.tile([128, 16], F32, tag="ps")
        nc.tensor.matmul(pf, xi[:, 128 * jc:128 * (jc + 1)], wia)
        nc.scalar.activation(out=et[:, jc, :], in_=pf, func=Act.Exp)
        nc.gpsimd.tensor_copy(et16[:, jc, :], et[:, jc, :])
        pwh = psum.tile([128, NHEADS * NHID], F32, tag="ps")
        nc.tensor.matmul(pwh, xb[:, 128 * jc:128 * (jc + 1)], wb)
        nc.scalar.copy(out=whp[:, jc, :, 0:NHID],
                       in_=pwh.rearrange("p (h o) -> p h o", h=NHEADS))
        csp = et16[:, jc, 6:12].rearrange("p (h o) -> p h o", o=1)
        cin = et16[:, jc, 14:16].rearrange("p (h o) -> p h o", o=1)
        nc.gpsimd.tensor_copy(whp[:, jc, 0:H_SP, NHID:NHID + 1], csp)
        nc.gpsimd.tensor_copy(whp[:, jc, H_SP:, NHID:NHID + 1], cin)
        nc.gpsimd.tensor_tensor(
            whp[:, jc, 0:H_SP, 0:NHID], whp[:, jc, 0:H_SP, 0:NHID],
            csp.to_broadcast([128, H_SP, NHID]), Alu.mult)
        nc.gpsimd.tensor_tensor(
            whp[:, jc, H_SP:, 0:NHID], whp[:, jc, H_SP:, 0:NHID],
            cin.to_broadcast([128, H_INT, NHID]), Alu.mult)
        # map + mask + accumulate; heads 6,7 use G = relu(P*R - 1) on Act
        # (w = C*(1+G) differs from C*max(P,Q) by a per-query factor that
        # cancels in softmax); the "+1" term is a plain adjacency matmul
        t = work.tile([128, NHEADS, R], BF16, tag="t")
        for h in range(4, NHEADS):
            pcol = h if h < H_SP else 12 + (h - H_SP)
            nc.scalar.activation(out=t[:, h, :], in_=qb[:, h, :], func=Act.Relu,
                                 scale=et[:, jc, pcol:pcol + 1],
                                 bias=negones[:, 0:1])
        for h in range(4):
            nc.vector.tensor_scalar(
                out=t[:, h, :], in0=qb[:, h, :],
                scalar1=et[:, jc, h:h + 1], scalar2=None, op0=Alu.max)
        m = work.tile([128, NHEADS, R], BF16, tag="m")
        adjb = adjT_sb[:, jc:jc + 1, :]
        nc.vector.tensor_tensor(
            m, t, adjb.to_broadcast([128, NHEADS, R]), Alu.mult)
        for h in range(NHEADS):
            nc.tensor.matmul(accs[h], whp[:, jc, h, :], m[:, h, :],
                             start=(jc == 0), stop=(jc == JT - 1 and h < 5))
        for h in range(4, NHEADS):
            nc.tensor.matmul(accs[h], whp[:, jc, h, :], adjT_sb[:, jc, :],
                             start=False, stop=(jc == JT - 1))

    # post-loop-only setup (kept off the startup critical path)
    aout_b = big.tile([NOUT, 2], BF16, tag="aout_b")
    nc.vector.tensor_copy(aout_b, aout_f)
    ident = big.tile([128, 128], F32, tag="ident")
    make_identity(nc, ident)
    ident_b = big.tile([128, 128], BF16, tag="ident_b")
    make_identity(nc, ident_b)
    ones1 = big.tile([1, 128], F32, tag="ones1")
    nc.vector.memset(ones1, 1.0)
    ones1_b = big.tile([1, 128], BF16, tag="ones1_b")
    nc.vector.memset(ones1_b, 1.0)

    # collective warmup: absorbs first-collective setup cost
    nc.gpsimd.collective_compute(
        "AllGather", Alu.bypass, replica_groups=[list(range(NCORES))],
        ins=[wag_in.opt()], outs=[wag_out.opt()])

    # ---------------- finalize: h = elu(num/den), feature-major ------------
    # asb pair-tile c: rows 0:33 = head 2c (num + den), 64:97 = head 2c+1
    asb = [big.tile([128, R], F32, tag=f"asb{c}", name=f"asb{c}")
           for c in range(4)]
    dens = big.tile([NHEADS, R], F32, tag="dens")
    for c in range(4):
        nc.vector.memset(asb[c], 0.0)
    for h in range(NHEADS):
        eng = nc.scalar if h % 2 == 0 else nc.vector
        if h % 2 == 0:
            nc.scalar.copy(
                out=asb[h // 2][64 * (h % 2):64 * (h % 2) + NHID + 1, :],
                in_=accs[h][:, :])
        else:
            nc.vector.tensor_copy(
                asb[h // 2][64 * (h % 2):64 * (h % 2) + NHID + 1, :],
                accs[h][:, :])
    ps_acc.close()
    ps_setup.close()
    for h in range(NHEADS):
        nc.gpsimd.dma_start(
            out=dens[h:h + 1, :],
            in_=asb[h // 2][64 * (h % 2) + NHID:64 * (h % 2) + NHID + 1, :])

    ps_post = ExitStack()
    psum = ps_post.enter_context(tc.tile_pool(name="ps_post", bufs=2,
                                              space="PSUM"))
    pfix = ps_post.enter_context(tc.tile_pool(name="pfix", bufs=1,
                                              space="PSUM"))

    rec8 = big.tile([NHEADS, R], F32, tag="rec8")
    nc.vector.reciprocal_approx_fast(out=rec8, in_=dens)
    hT = [big.tile([128, R], BF16, tag=f"hT{c}", name=f"hT{c}")
          for c in range(4)]
    for c in range(4):
        recb = psum.tile([128, R], F32, tag="ps")
        nc.tensor.matmul(recb, esel[:, 128 * c:128 * (c + 1)], rec8)
        v = ext.tile([128, R], F32, tag="v")
        nc.vector.tensor_tensor(v, asb[c], recb, Alu.mult)
        e = ext.tile([128, R], F32, tag="e")
        nc.scalar.activation(out=e, in_=v, func=Act.Exp)
        em1 = ext.tile([128, R], F32, tag="em1")
        nc.vector.tensor_scalar(out=em1, in0=e, scalar1=-1.0, scalar2=None,
                                op0=Alu.add)
        r = ext.tile([128, R], F32, tag="r")
        nc.vector.tensor_scalar(out=r, in0=v, scalar1=0.0, scalar2=None,
                                op0=Alu.max)
        nc.vector.tensor_tensor(hT[c], em1, r, Alu.min)

    # ---------------- Who (feature-major), o1/o2, qob ----------------------
    pwho = pfix.tile([NOUT, R], F32, tag="pwho")
    for c in range(4):
        nc.tensor.matmul(pwho, w4sb[:, c, :], hT[c], start=(c == 0),
                         stop=(c == 3))
    whoT_b = big.tile([NOUT, R], BF16, tag="whoT_b")
    nc.scalar.copy(out=whoT_b, in_=pwho)
    po1 = psum.tile([1, R], F32, tag="ps")
    nc.tensor.matmul(po1, aout_b[:, 0:1], whoT_b)
    po2 = psum.tile([1, R], F32, tag="ps")
    nc.tensor.matmul(po2, aout_b[:, 1:2], whoT_b)
    ro_sb = big.tile([1, R], BF16, tag="ro_sb")
    nc.scalar.activation(out=ro_sb, in_=po1, func=Act.Exp, scale=0.8)
    co_sb = big.tile([1, R], BF16, tag="co_sb")
    nc.scalar.activation(out=co_sb, in_=po2, func=Act.Exp, scale=0.2)
    po_sb = big.tile([1, R], BF16, tag="po_sb")
    nc.scalar.activation(out=po_sb, in_=po2, func=Act.Exp, scale=0.8)
    prob = psum.tile([128, R], F32, tag="ps")
    nc.tensor.matmul(prob, ones1_b, ro_sb)
    rob = big.tile([128, R], BF16, tag="rob")
    nc.scalar.copy(out=rob, in_=prob)


    # ---------------- payload [R, 66] built transposed ---------------------
    # cols 0:64 = C_o*Who, 64 = C_o, 65 = P_o
    cob = psum.tile([NOUT, R], F32, tag="ps")
    nc.tensor.matmul(cob, ones1_b[0:1, 0:NOUT], co_sb)
    whoc = big.tile([NOUT, R], BF16, tag="whoc")
    nc.vector.tensor_tensor(whoc, whoT_b, cob, Alu.mult)
    payT = big.tile([128, 4, NOUT + 2], BF16, tag="payT")
    for k in range(4):
        ppt = psum.tile([128, NOUT + 4], BF16, tag="psb")
        sl = slice(128 * k, 128 * (k + 1))
        nc.tensor.transpose(ppt[:, 0:NOUT], whoc[:, sl], ident_b[0:NOUT, 0:NOUT])
        nc.tensor.transpose(ppt[:, NOUT:NOUT + 1], co_sb[:, sl],
                            ident_b[0:1, 0:1])
        nc.tensor.transpose(ppt[:, NOUT + 2:NOUT + 3], po_sb[:, sl],
                            ident_b[0:1, 0:1])
        nc.scalar.copy(out=payT[:, k, 0:NOUT + 1], in_=ppt[:, 0:NOUT + 1])
        nc.scalar.copy(out=payT[:, k, NOUT + 1:NOUT + 2],
                       in_=ppt[:, NOUT + 2:NOUT + 3])
    ccin = dram.tile([128, 4 * (NOUT + 2)], BF16, tag="ccin")
    ccout = dram.tile([NCORES * 128, 4 * (NOUT + 2)], BF16, tag="ccout")
    nc.sync.dma_start(out=ccin.rearrange("p (k c) -> p k c", k=4), in_=payT)
    nc.gpsimd.collective_compute(
        "AllGather", Alu.bypass, replica_groups=[list(range(NCORES))],
        ins=[ccin.opt()], outs=[ccout.opt()])
    # ---------------- output attention (flipped) ---------------------------
    whop = big.tile([128, JT, NOUT + 2], BF16, tag="whop")
    for g in range(NCORES):
        nc.sync.dma_start(
            out=whop[:, 4 * g:4 * (g + 1), :],
            in_=ccout[128 * g:128 * (g + 1), :].rearrange("p (k c) -> p k c", k=4))
    pco = big.tile([128, JT], F32, tag="pco")
    nc.scalar.copy(out=pco, in_=whop[:, :, NOUT + 1])
    acc2 = pfix.tile([NOUT + 1, R], F32, tag="acc2")
    for jc in range(JT):
        t2 = work.tile([128, R], BF16, tag="t2")
        if jc % 2 == 1:
            nc.scalar.activation(out=t2, in_=rob, func=Act.Relu,
                                 scale=pco[:, jc:jc + 1], bias=negones[:, 0:1])
        else:
            nc.vector.tensor_scalar(out=t2, in0=rob,
                                    scalar1=pco[:, jc:jc + 1], scalar2=1.0,
                                    op0=Alu.mult, op1=Alu.max)
        m2 = work.tile([128, R], BF16, tag="m2")
        nc.vector.tensor_tensor(m2, t2, adjT_sb[:, jc, :], Alu.mult)
        nc.tensor.matmul(acc2, whop[:, jc, 0:NOUT + 1], m2,
                         start=(jc == 0), stop=False)
        if jc % 2 == 1:
            nc.tensor.matmul(acc2, whop[:, jc, 0:NOUT + 1], adjT_sb[:, jc, :],
                             start=False, stop=(jc == JT - 1))

    # ---------------- out = tanh(num/den), transpose back ------------------
    den2 = big.tile([1, R], F32, tag="den2")
    nc.scalar.copy(out=den2, in_=acc2[NOUT:NOUT + 1, :])
    rec2 = big.tile([1, R], F32, tag="rec2")
    nc.vector.reciprocal_approx_fast(out=rec2, in_=den2)
    recb2 = psum.tile([NOUT, R], F32, tag="ps")
    nc.tensor.matmul(recb2, ones1[0:1, 0:NOUT], rec2)
    nsb = big.tile([NOUT, R], F32, tag="nsb")
    nc.scalar.copy(out=nsb, in_=acc2[0:NOUT, :])
    sc = big.tile([NOUT, R], F32, tag="sc")
    nc.vector.tensor_tensor(sc, nsb, recb2, Alu.mult)
    outT = big.tile([NOUT, R], F32, tag="outT")
    nc.scalar.activation(out=outT, in_=sc, func=Act.Tanh)
    out_sb = big.tile([128, 4, NOUT], F32, tag="out_sb")
    for k in range(4):
        pot = psum.tile([128, NOUT], F32, tag="ps")
        nc.tensor.transpose(pot, outT[:, 128 * k:128 * (k + 1)],
                            ident[0:NOUT, 0:NOUT])
        nc.scalar.copy(out=out_sb[:, k, :], in_=pot)
    nc.sync.dma_start(out=d_out.ap().rearrange("(k p) c -> p k c", p=128),
                      in_=out_sb)
    ps_post.close()


_NC_CACHE = None


def _get_nc():
    global _NC_CACHE
    if _NC_CACHE is None:
        _NC_CACHE = _build_program()
    return _NC_CACHE


def _make_in_maps(inputs):
    x = np.asarray(inputs["x"], np.float32)
    adj = np.asarray(inputs["adj"], np.float32)
    ie = np.asarray(inputs["intent_embeds"], np.float32)
    w_sp = np.asarray(inputs["W_sp"], np.float32)
    w_int = np.asarray(inputs["W_int"], np.float32)
    w_out = np.asarray(inputs["W_out"], np.float32)
    xT_full = np.ascontiguousarray(x.T)
    xb_full = xT_full.astype(ml_dtypes.bfloat16)
    xi_full = np.ascontiguousarray(np.concatenate([x, ie], axis=1).T)
    # wb: [NIN, NHEADS*NHID], head-major cols, spatial then intent
    wb = np.concatenate([w_sp.transpose(1, 0, 2).reshape(NIN, -1),
                         w_int.transpose(1, 0, 2).reshape(NIN, -1)],
                        axis=1).astype(ml_dtypes.bfloat16)
    a_sp = np.asarray(inputs["a_sp"], np.float32)
    a_int = np.asarray(inputs["a_int"], np.float32)
    # wia rows 0:64 = [0.8*W@a2 | 0.2*W@a2] (spatial), rows 64:96 = intent
    w2 = np.einsum('hfo,ho->fh', w_sp, a_sp[:, NHID:])        # [64, 6]
    w1 = np.einsum('hfo,ho->fh', w_sp, a_sp[:, :NHID])        # [64, 6]
    wia = np.zeros((NIN + D_INT, 16), np.float32)
    wia[0:NIN, 0:H_SP] = 0.8 * w2
    wia[0:NIN, H_SP:2 * H_SP] = 0.2 * w2
    wia[NIN:, 12:12 + H_INT] = 0.8 * a_int[:, D_INT:].T
    wia[NIN:, 12 + H_INT:16] = 0.2 * a_int[:, D_INT:].T
    # wqa cols: 0:6 = -0.8*w1 (Q rows, spatial), 6 = +0.8*w1[5], 7 = +0.8*w1[4]
    # (R rows for G-form heads), 8:10 = +0.8*a1 (intent R rows)
    wqa = np.zeros((NIN + D_INT, 10), np.float32)
    wqa[0:NIN, 0:H_SP] = -0.8 * w1
    wqa[0:NIN, 6] = 0.8 * w1[:, 5]
    wqa[0:NIN, 7] = 0.8 * w1[:, 4]
    wqa[NIN:, 8:10] = 0.8 * a_int[:, :D_INT].T
    wqa = wqa.astype(ml_dtypes.bfloat16)
    rowfor = {0: 0, 1: 1, 2: 2, 3: 3, 4: 7, 5: 6, 6: 8, 7: 9}
    qsel = np.zeros((10, NHEADS * 128), np.float32)
    for h in range(NHEADS):
        qsel[rowfor[h], 128 * h:128 * (h + 1)] = 1.0
    qsel = qsel.astype(ml_dtypes.bfloat16)
    # w4: pair-tile chunks [4, 128, NOUT]; rows 0:32 head 2c, 64:96 head 2c+1
    w4 = np.zeros((4, 128, NOUT), np.float32)
    for c in range(4):
        w4[c, 0:NHID] = w_out[NHID * 2 * c:NHID * (2 * c) + NHID]
        w4[c, 64:64 + NHID] = w_out[NHID * (2 * c + 1):NHID * (2 * c + 1) + NHID]
    w4 = w4.astype(ml_dtypes.bfloat16)
    esel = np.zeros((NHEADS, R), np.float32)
    for c in range(4):
        esel[2 * c, 128 * c:128 * c + 64] = 1.0
        esel[2 * c + 1, 128 * c + 64:128 * (c + 1)] = 1.0
    in_maps = []
    for d in range(NCORES):
        sl = slice(d * R, (d + 1) * R)
        in_maps.append({
            "xi": xi_full, "xb": xb_full,
            "adjT": np.ascontiguousarray(
                adj[sl, :].T.reshape(JT, 128, R).transpose(1, 0, 2)
                .reshape(128, JT * R)).astype(ml_dtypes.bfloat16),
            "xoT": np.ascontiguousarray(x[sl].T).astype(ml_dtypes.bfloat16),
            "ioT": np.ascontiguousarray(ie[sl].T).astype(ml_dtypes.bfloat16),
            "wqa": wqa, "qsel": qsel,
            "wia": wia, "wb": wb, "w4": w4,
            "aout": np.asarray(inputs["a_out"], np.float32),
            "esel": esel,
        })
    return in_maps


def kernel(x, adj, intent_embeds, W_sp, a_sp, W_int, a_int, W_out, a_out):
    nc = _get_nc()
    in_maps = _make_in_maps(dict(
        x=x, adj=adj, intent_embeds=intent_embeds, W_sp=W_sp, a_sp=a_sp,
        W_int=W_int, a_int=a_int, W_out=W_out, a_out=a_out))
    res = run_bass_kernel_spmd(nc, in_maps, list(range(NCORES)))
    return np.concatenate([res.results[d]["out"] for d in range(NCORES)], axis=0)


# revision 38
# speedup vs baseline: 1.2473x; 1.2473x over previous
"""Trainium2 Bass kernel for nn_GAT_7507602833557 (8-core SPMD GAT).

Sharding: query-node rows split across 8 cores (512 rows each); keys/values
replicated. Per-core adjacency passed pre-transposed ([keys, own queries]) in
bf16 ({0,1} exact).

Math (per attention map): exp(lrelu(f1[i]+f2[j])) = e^{f1}·C[j]·max(P[j],Q[i])
with P=e^{0.8 f2}, C=e^{0.2 f2}, Q=e^{-0.8 f1}; e^{f1} cancels in softmax.
C is folded into the value matrix (whp, incl. its ones column -> denominator),
so the map is one single-op tensor_scalar (max) + one mask multiply.

Layer-1 matmuls run "flipped": stationary = whp [128k, 33], moving = the
masked map m [128k, 512q], accumulating [33, 512] per head across 32 key
tiles into 8 rotating PSUM banks. Result is feature-major, which feeds the
output-layer W_out matmul directly (no transposes); softmax division uses
reciprocal_approx_fast + a PE ones-broadcast.
"""
import numpy as np

import concourse.bass as bass
import concourse.bacc as bacc
import concourse.tile as tile
from concourse import mybir
from concourse.bass_utils import run_bass_kernel_spmd
from concourse.masks import make_identity

import ml_dtypes

N, NIN, NHID, NOUT = 4096, 64, 32, 64
NHEADS, D_INT = 8, 32
H_SP, H_INT = 6, 2
NCORES = 8
R = N // NCORES           # 512 own query rows per core
JT = N // 128             # 32 key tiles
F32 = mybir.dt.float32
BF16 = mybir.dt.bfloat16
POOL_HEADS = 2            # trailing heads whose mask-mult runs on gpsimd
L2_POOL_STRIDE = 4        # every 4th output-layer mask-mult on gpsimd


def _build_program():
    nc = bacc.Bacc("TRN2", target_bir_lowering=False, debug=False,
                   num_devices=NCORES)
    d_xi = nc.dram_tensor("xi", [NIN + D_INT, N], F32, kind="ExternalInput")
    d_xb = nc.dram_tensor("xb", [NIN, N], BF16, kind="ExternalInput")
    d_adjT = nc.dram_tensor("adjT", [128, JT * R], BF16, kind="ExternalInput")
    d_xo = nc.dram_tensor("xoT", [NIN, R], BF16, kind="ExternalInput")
    d_io = nc.dram_tensor("ioT", [D_INT, R], BF16, kind="ExternalInput")
    d_wia = nc.dram_tensor("wia", [NIN + D_INT, 16], F32, kind="ExternalInput")
    d_wqa = nc.dram_tensor("wqa", [NIN + D_INT, 10], BF16, kind="ExternalInput")
    d_qsel = nc.dram_tensor("qsel", [10, NHEADS * 128], BF16, kind="ExternalInput")
    d_wb = nc.dram_tensor("wb", [NIN, NHEADS * NHID], BF16, kind="ExternalInput")
    d_w4 = nc.dram_tensor("w4", [4, 128, NOUT], BF16, kind="ExternalInput")
    d_aout = nc.dram_tensor("aout", [2 * NOUT], F32, kind="ExternalInput")
    d_esel = nc.dram_tensor("esel", [NHEADS, R], F32, kind="ExternalInput")
    d_out = nc.dram_tensor("out", [R, NOUT], F32, kind="ExternalOutput")

    with tile.TileContext(nc) as tc:
        _kernel_body(tc, d_xi, d_xb, d_adjT, d_xo, d_io, d_wia, d_wqa,
                     d_qsel, d_wb, d_w4, d_aout, d_esel, d_out)
    nc.compile()
    return nc


def _kernel_body(tc, d_xi, d_xb, d_adjT, d_xo, d_io, d_wia, d_wqa, d_qsel,
                 d_wb, d_w4, d_aout, d_esel, d_out):
    nc = tc.nc
    Act = mybir.ActivationFunctionType
    Alu = mybir.AluOpType
    from contextlib import ExitStack

    ctx = ExitStack()
    big = ctx.enter_context(tc.tile_pool(name="big", bufs=1))
    work = ctx.enter_context(tc.tile_pool(name="work", bufs=2))
    ext = ctx.enter_context(tc.tile_pool(name="ext", bufs=2))
    dram = ctx.enter_context(tc.tile_pool(name="dram", bufs=1, space="DRAM"))
    try:
        _body(tc, ctx, big, work, ext, dram, d_xi, d_xb, d_adjT, d_xo,
              d_io, d_wia, d_wqa, d_qsel, d_wb, d_w4, d_aout, d_esel, d_out)
    finally:
        ctx.close()


def _body(tc, ctx, big, work, ext, dram, d_xi, d_xb, d_adjT, d_xo, d_io,
          d_wia, d_wqa, d_qsel, d_wb, d_w4, d_aout, d_esel, d_out):
    nc = tc.nc
    Act = mybir.ActivationFunctionType
    Alu = mybir.AluOpType
    from contextlib import ExitStack

    # ---------------- loads (critical-path-first order) --------------------
    wqa = big.tile([NIN + D_INT, 10], BF16, tag="wqa")
    nc.sync.dma_start(out=wqa, in_=d_wqa.ap())
    qsel = big.tile([10, NHEADS * 128], BF16, tag="qsel")
    nc.sync.dma_start(out=qsel, in_=d_qsel.ap())
    xio = big.tile([NIN + D_INT, R], BF16, tag="xio")
    nc.sync.dma_start(out=xio[0:NIN, :], in_=d_xo.ap())
    nc.sync.dma_start(out=xio[NIN:NIN + D_INT, :], in_=d_io.ap())
    wia = big.tile([NIN + D_INT, 16], F32, tag="wia")
    nc.sync.dma_start(out=wia, in_=d_wia.ap())
    xi = big.tile([NIN + D_INT, N], F32, tag="xi")
    nc.sync.dma_start(out=xi[:, 0:1024], in_=d_xi.ap()[:, 0:1024])
    wb = big.tile([NIN, NHEADS * NHID], BF16, tag="wb")
    nc.sync.dma_start(out=wb, in_=d_wb.ap())
    adjT_sb = big.tile([128, JT, R], BF16, tag="adjT_sb")
    nc.sync.dma_start(
        out=adjT_sb[:, 0:4, :],
        in_=d_adjT.ap()[:, 0:4 * R].rearrange("p (t i) -> p t i", i=R))
    xb = big.tile([NIN, N], BF16, tag="xb")
    nc.sync.dma_start(out=xb, in_=d_xb.ap())
    for g in range(1, 4):
        nc.sync.dma_start(
            out=xi[:, 1024 * g:1024 * (g + 1)],
            in_=d_xi.ap()[:, 1024 * g:1024 * (g + 1)])
    for g in range(1, 8):
        nc.sync.dma_start(
            out=adjT_sb[:, 4 * g:4 * (g + 1), :],
            in_=d_adjT.ap()[:, 4 * g * R:4 * (g + 1) * R]
                .rearrange("p (t i) -> p t i", i=R))
    w4sb = big.tile([128, 4, NOUT], BF16, tag="w4sb")
    nc.sync.dma_start(out=w4sb, in_=d_w4.ap().rearrange("c p o -> p c o"))
    esel = big.tile([NHEADS, R], F32, tag="esel")
    nc.sync.dma_start(out=esel, in_=d_esel.ap())
    aout_f = big.tile([NOUT, 2], F32, tag="aout_f")
    nc.sync.dma_start(out=aout_f, in_=d_aout.ap().rearrange("(c o) -> o c", c=2))

    ps_setup = ExitStack()
    psum = ps_setup.enter_context(tc.tile_pool(name="ps_setup", bufs=3,
                                               space="PSUM"))

    dummy_act = big.tile([1, 2], F32, tag="dummy_act")
    nc.vector.memset(dummy_act, 0.5)
    nc.scalar.activation(out=dummy_act, in_=dummy_act, func=Act.Exp)

    # -------- Q/R rows (one stacked matmul) + qb broadcast via selector ----
    pq = psum.tile([10, R], F32, tag="ps")
    nc.tensor.matmul(pq, wqa, xio)
    qrow = big.tile([10, R], BF16, tag="qrow")
    nc.scalar.activation(out=qrow, in_=pq, func=Act.Exp)
    negones = big.tile([128, 1], F32, tag="negones")
    nc.vector.memset(negones, -1.0)
    qb = big.tile([128, NHEADS, R], BF16, tag="qb")
    for h in [4, 5, 6, 7, 0, 1, 2, 3]:
        pqb = psum.tile([128, R], F32, tag="ps")
        nc.tensor.matmul(pqb, qsel[:, 128 * h:128 * (h + 1)], qrow)
        nc.vector.tensor_copy(qb[:, h, :], pqb)

    # ------- layer-1 attention; ET/whp production interleaved per jc -------
    # et cols 0:6 P_sp, 6:12 C_sp, 12:14 P_int, 14:16 C_int
    et = big.tile([128, JT, 16], F32, tag="et")
    et16 = big.tile([128, JT, 16], BF16, tag="et16")
    whp = big.tile([128, JT, NHEADS, NHID + 1], BF16, tag="whp")
    ps_acc = ExitStack()
    pacc = ps_acc.enter_context(tc.tile_pool(name="pacc", bufs=1, space="PSUM"))
    apk = [pacc.tile([64 + NHID + 1, R], F32, tag=f"apk{c}", name=f"apk{c}")
           for c in range(4)]
    accs = [apk[h // 2][64 * (h % 2):64 * (h % 2) + NHID + 1, :]
            for h in range(NHEADS)]
    HORDER = [6, 7, 0, 1, 2, 3, 4, 5]
    wag_in = dram.tile([16, 2], BF16, tag="wag_in")
    wag_out = dram.tile([128, 2], BF16, tag="wag_out")
    nc.sync.dma_start(out=wag_in, in_=d_adjT.ap()[0:16, 0:2])
    for jc in range(JT):
        # production for this key tile (runs ahead on tensor/scalar/gpsimd)
        pf = psum.tile([128, 16], F32, tag="ps")
        nc.tensor.matmul(pf, xi[:, 128 * jc:128 * (jc + 1)], wia)
        nc.scalar.activation(out=et[:, jc, :], in_=pf, func=Act.Exp)
        nc.gpsimd.tensor_copy(et16[:, jc, :], et[:, jc, :])
        pwh = psum.tile([128, NHEADS * NHID], F32, tag="ps")
        nc.tensor.matmul(pwh, xb[:, 128 * jc:128 * (jc + 1)], wb)
        nc.scalar.copy(out=whp[:, jc, :, 0:NHID],
                       in_=pwh.rearrange("p (h o) -> p h o", h=NHEADS))
        csp = et16[:, jc, 6:12].rearrange("p (h o) -> p h o", o=1)
        cin = et16[:, jc, 14:16].rearrange("p (h o) -> p h o", o=1)
        nc.gpsimd.tensor_copy(whp[:, jc, 0:H_SP, NHID:NHID + 1], csp)
        nc.gpsimd.tensor_copy(whp[:, jc, H_SP:, NHID:NHID + 1], cin)
        nc.gpsimd.tensor_tensor(
            whp[:, jc, 0:H_SP, 0:NHID], whp[:, jc, 0:H_SP, 0:NHID],
            csp.to_broadcast([128, H_SP, NHID]), Alu.mult)
        nc.gpsimd.tensor_tensor(
            whp[:, jc, H_SP:, 0:NHID], whp[:, jc, H_SP:, 0:NHID],
            cin.to_broadcast([128, H_INT, NHID]), Alu.mult)
        # map + mask + accumulate; heads 6,7 use G = relu(P*R - 1) on Act
        # (w = C*(1+G) differs from C*max(P,Q) by a per-query factor that
        # cancels in softmax); the "+1" term is a plain adjacency matmul
        t = work.tile([128, NHEADS, R], BF16, tag="t")
        for h in range(4, NHEADS):
            pcol = h if h < H_SP else 12 + (h - H_SP)
            nc.scalar.activation(out=t[:, h, :], in_=qb[:, h, :], func=Act.Relu,
                                 scale=et[:, jc, pcol:pcol + 1],
                                 bias=negones[:, 0:1])
        for h in range(4):
            nc.vector.tensor_scalar(
                out=t[:, h, :], in0=qb[:, h, :],
                scalar1=et[:, jc, h:h + 1], scalar2=None, op0=Alu.max)
        m = work.tile([128, NHEADS, R], BF16, tag="m")
        adjb = adjT_sb[:, jc:jc + 1, :]
        nc.vector.tensor_tensor(
            m, t, adjb.to_broadcast([128, NHEADS, R]), Alu.mult)
        for h in range(NHEADS):
            nc.tensor.matmul(accs[h], whp[:, jc, h, :], m[:, h, :],
                             start=(jc == 0), stop=(jc == JT - 1 and h < 5))
        for h in range(4, NHEADS):
            nc.tensor.matmul(accs[h], whp[:, jc, h, :], adjT_sb[:, jc, :],
                             start=False, stop=(jc == JT - 1))

    # post-loop-only setup (kept off the startup critical path)
    aout_b = big.tile([NOUT, 2], BF16, tag="aout_b")
    nc.vector.tensor_copy(aout_b, aout_f)
    ident = big.tile([128, 128], F32, tag="ident")
    make_identity(nc, ident)
    ident_b = big.tile([128, 128], BF16, tag="ident_b")
    make_identity(nc, ident_b)
    ones1 = big.tile([1, 128], F32, tag="ones1")
    nc.vector.memset(ones1, 1.0)
    ones1_b = big.tile([1, 128], BF16, tag="ones1_b")
    nc.vector.memset(ones1_b, 1.0)

    # collective warmup: absorbs first-collective setup cost
    nc.gpsimd.collective_compute(
        "AllGather", Alu.bypass, replica_groups=[list(range(NCORES))],
        ins=[wag_in.opt()], outs=[wag_out.opt()])

    # ---------------- finalize: h = elu(num/den), feature-major ------------
    # asb pair-tile c: rows 0:33 = head 2c (num + den), 64:97 = head 2c+1
    asb = [big.tile([128, R], F32, tag=f"asb{c}", name=f"asb{c}")
           for c in range(4)]
    dens = big.tile([NHEADS, R], F32, tag="dens")
    for c in range(4):
        nc.vector.memset(asb[c], 0.0)
    for h in range(NHEADS):
        nc.scalar.copy(out=asb[h // 2][64 * (h % 2):64 * (h % 2) + NHID + 1, :],
                       in_=accs[h][:, :])
    ps_acc.close()
    ps_setup.close()
    for h in range(NHEADS):
        nc.gpsimd.dma_start(
            out=dens[h:h + 1, :],
            in_=asb[h // 2][64 * (h % 2) + NHID:64 * (h % 2) + NHID + 1, :])

    ps_post = ExitStack()
    psum = ps_post.enter_context(tc.tile_pool(name="ps_post", bufs=2,
                                              space="PSUM"))
    pfix = ps_post.enter_context(tc.tile_pool(name="pfix", bufs=1,
                                              space="PSUM"))

    rec8 = big.tile([NHEADS, R], F32, tag="rec8")
    nc.vector.reciprocal_approx_fast(out=rec8, in_=dens)
    hT = [big.tile([128, R], BF16, tag=f"hT{c}", name=f"hT{c}")
          for c in range(4)]
    for c in range(4):
        recb = psum.tile([128, R], F32, tag="ps")
        nc.tensor.matmul(recb, esel[:, 128 * c:128 * (c + 1)], rec8)
        v = ext.tile([128, R], F32, tag="v")
        nc.vector.tensor_tensor(v, asb[c], recb, Alu.mult)
        e = ext.tile([128, R], F32, tag="e")
        nc.scalar.activation(out=e, in_=v, func=Act.Exp)
        em1 = ext.tile([128, R], F32, tag="em1")
        nc.vector.tensor_scalar(out=em1, in0=e, scalar1=-1.0, scalar2=None,
                                op0=Alu.add)
        r = ext.tile([128, R], F32, tag="r")
        nc.vector.tensor_scalar(out=r, in0=v, scalar1=0.0, scalar2=None,
                                op0=Alu.max)
        nc.vector.tensor_tensor(hT[c], em1, r, Alu.min)

    # ---------------- Who (feature-major), o1/o2, qob ----------------------
    pwho = pfix.tile([NOUT, R], F32, tag="pwho")
    for c in range(4):
        nc.tensor.matmul(pwho, w4sb[:, c, :], hT[c], start=(c == 0),
                         stop=(c == 3))
    whoT_b = big.tile([NOUT, R], BF16, tag="whoT_b")
    nc.scalar.copy(out=whoT_b, in_=pwho)
    po1 = psum.tile([1, R], F32, tag="ps")
    nc.tensor.matmul(po1, aout_b[:, 0:1], whoT_b)
    po2 = psum.tile([1, R], F32, tag="ps")
    nc.tensor.matmul(po2, aout_b[:, 1:2], whoT_b)
    qo_sb = big.tile([1, R], BF16, tag="qo_sb")
    nc.scalar.activation(out=qo_sb, in_=po1, func=Act.Exp, scale=-0.8)
    co_sb = big.tile([1, R], BF16, tag="co_sb")
    nc.scalar.activation(out=co_sb, in_=po2, func=Act.Exp, scale=0.2)
    po_sb = big.tile([1, R], BF16, tag="po_sb")
    nc.scalar.activation(out=po_sb, in_=po2, func=Act.Exp, scale=0.8)
    pqob = psum.tile([128, R], F32, tag="ps")
    nc.tensor.matmul(pqob, ones1_b, qo_sb)
    qob = big.tile([128, R], BF16, tag="qob")
    nc.scalar.copy(out=qob, in_=pqob)


    # ---------------- payload [R, 66] built transposed ---------------------
    # cols 0:64 = C_o*Who, 64 = C_o, 65 = P_o
    cob = psum.tile([NOUT, R], F32, tag="ps")
    nc.tensor.matmul(cob, ones1_b[0:1, 0:NOUT], co_sb)
    whoc = big.tile([NOUT, R], BF16, tag="whoc")
    nc.vector.tensor_tensor(whoc, whoT_b, cob, Alu.mult)
    payT = big.tile([128, 4, NOUT + 2], BF16, tag="payT")
    for k in range(4):
        ppt = psum.tile([128, NOUT + 4], BF16, tag="psb")
        sl = slice(128 * k, 128 * (k + 1))
        nc.tensor.transpose(ppt[:, 0:NOUT], whoc[:, sl], ident_b[0:NOUT, 0:NOUT])
        nc.tensor.transpose(ppt[:, NOUT:NOUT + 1], co_sb[:, sl],
                            ident_b[0:1, 0:1])
        nc.tensor.transpose(ppt[:, NOUT + 2:NOUT + 3], po_sb[:, sl],
                            ident_b[0:1, 0:1])
        nc.scalar.copy(out=payT[:, k, 0:NOUT + 1], in_=ppt[:, 0:NOUT + 1])
        nc.scalar.copy(out=payT[:, k, NOUT + 1:NOUT + 2],
                       in_=ppt[:, NOUT + 2:NOUT + 3])
    ccin = dram.tile([128, 4 * (NOUT + 2)], BF16, tag="ccin")
    ccout = dram.tile([NCORES * 128, 4 * (NOUT + 2)], BF16, tag="ccout")
    nc.sync.dma_start(out=ccin.rearrange("p (k c) -> p k c", k=4), in_=payT)
    nc.gpsimd.collective_compute(
        "AllGather", Alu.bypass, replica_groups=[list(range(NCORES))],
        ins=[ccin.opt()], outs=[ccout.opt()])
    # ---------------- output attention (flipped) ---------------------------
    whop = big.tile([128, JT, NOUT + 2], BF16, tag="whop")
    for g in range(NCORES):
        nc.sync.dma_start(
            out=whop[:, 4 * g:4 * (g + 1), :],
            in_=ccout[128 * g:128 * (g + 1), :].rearrange("p (k c) -> p k c", k=4))
    pco = big.tile([128, JT], F32, tag="pco")
    nc.scalar.copy(out=pco, in_=whop[:, :, NOUT + 1])
    acc2 = pfix.tile([NOUT + 1, R], F32, tag="acc2")
    for jc in range(JT):
        t2 = work.tile([128, R], BF16, tag="t2")
        nc.vector.tensor_scalar(out=t2, in0=qob, scalar1=pco[:, jc:jc + 1],
                                scalar2=None, op0=Alu.max)
        m2 = work.tile([128, R], BF16, tag="m2")
        nc.vector.tensor_tensor(m2, t2, adjT_sb[:, jc, :], Alu.mult)
        nc.tensor.matmul(acc2, whop[:, jc, 0:NOUT + 1], m2,
                         start=(jc == 0), stop=(jc == JT - 1))

    # ---------------- out = tanh(num/den), transpose back ------------------
    den2 = big.tile([1, R], F32, tag="den2")
    nc.scalar.copy(out=den2, in_=acc2[NOUT:NOUT + 1, :])
    rec2 = big.tile([1, R], F32, tag="rec2")
    nc.vector.reciprocal_approx_fast(out=rec2, in_=den2)
    recb2 = psum.tile([NOUT, R], F32, tag="ps")
    nc.tensor.matmul(recb2, ones1[0:1, 0:NOUT], rec2)
    nsb = big.tile([NOUT, R], F32, tag="nsb")
    nc.scalar.copy(out=nsb, in_=acc2[0:NOUT, :])
    sc = big.tile([NOUT, R], F32, tag="sc")
    nc.vector.tensor_tensor(sc, nsb, recb2, Alu.mult)
    outT = big.tile([NOUT, R], F32, tag="outT")
    nc.scalar.activation(out=outT, in_=sc, func=Act.Tanh)
    out_sb = big.tile([128, 4, NOUT], F32, tag="out_sb")
    for k in range(4):
        pot = psum.tile([128, NOUT], F32, tag="ps")
        nc.tensor.transpose(pot, outT[:, 128 * k:128 * (k + 1)],
                            ident[0:NOUT, 0:NOUT])
        nc.scalar.copy(out=out_sb[:, k, :], in_=pot)
    nc.sync.dma_start(out=d_out.ap().rearrange("(k p) c -> p k c", p=128),
                      in_=out_sb)
    ps_post.close()


_NC_CACHE = None


def _get_nc():
    global _NC_CACHE
    if _NC_CACHE is None:
        _NC_CACHE = _build_program()
    return _NC_CACHE


def _make_in_maps(inputs):
    x = np.asarray(inputs["x"], np.float32)
    adj = np.asarray(inputs["adj"], np.float32)
    ie = np.asarray(inputs["intent_embeds"], np.float32)
    w_sp = np.asarray(inputs["W_sp"], np.float32)
    w_int = np.asarray(inputs["W_int"], np.float32)
    w_out = np.asarray(inputs["W_out"], np.float32)
    xT_full = np.ascontiguousarray(x.T)
    xb_full = xT_full.astype(ml_dtypes.bfloat16)
    xi_full = np.ascontiguousarray(np.concatenate([x, ie], axis=1).T)
    # wb: [NIN, NHEADS*NHID], head-major cols, spatial then intent
    wb = np.concatenate([w_sp.transpose(1, 0, 2).reshape(NIN, -1),
                         w_int.transpose(1, 0, 2).reshape(NIN, -1)],
                        axis=1).astype(ml_dtypes.bfloat16)
    a_sp = np.asarray(inputs["a_sp"], np.float32)
    a_int = np.asarray(inputs["a_int"], np.float32)
    # wia rows 0:64 = [0.8*W@a2 | 0.2*W@a2] (spatial), rows 64:96 = intent
    w2 = np.einsum('hfo,ho->fh', w_sp, a_sp[:, NHID:])        # [64, 6]
    w1 = np.einsum('hfo,ho->fh', w_sp, a_sp[:, :NHID])        # [64, 6]
    wia = np.zeros((NIN + D_INT, 16), np.float32)
    wia[0:NIN, 0:H_SP] = 0.8 * w2
    wia[0:NIN, H_SP:2 * H_SP] = 0.2 * w2
    wia[NIN:, 12:12 + H_INT] = 0.8 * a_int[:, D_INT:].T
    wia[NIN:, 12 + H_INT:16] = 0.2 * a_int[:, D_INT:].T
    # wqa cols: 0:6 = -0.8*w1 (Q rows, spatial), 6 = +0.8*w1[5], 7 = +0.8*w1[4]
    # (R rows for G-form heads), 8:10 = +0.8*a1 (intent R rows)
    wqa = np.zeros((NIN + D_INT, 10), np.float32)
    wqa[0:NIN, 0:H_SP] = -0.8 * w1
    wqa[0:NIN, 6] = 0.8 * w1[:, 5]
    wqa[0:NIN, 7] = 0.8 * w1[:, 4]
    wqa[NIN:, 8:10] = 0.8 * a_int[:, :D_INT].T
    wqa = wqa.astype(ml_dtypes.bfloat16)
    rowfor = {0: 0, 1: 1, 2: 2, 3: 3, 4: 7, 5: 6, 6: 8, 7: 9}
    qsel = np.zeros((10, NHEADS * 128), np.float32)
    for h in range(NHEADS):
        qsel[rowfor[h], 128 * h:128 * (h + 1)] = 1.0
    qsel = qsel.astype(ml_dtypes.bfloat16)
    # w4: pair-tile chunks [4, 128, NOUT]; rows 0:32 head 2c, 64:96 head 2c+1
    w4 = np.zeros((4, 128, NOUT), np.float32)
    for c in range(4):
        w4[c, 0:NHID] = w_out[NHID * 2 * c:NHID * (2 * c) + NHID]
        w4[c, 64:64 + NHID] = w_out[NHID * (2 * c + 1):NHID * (2 * c + 1) + NHID]
    w4 = w4.astype(ml_dtypes.bfloat16)
    esel = np.zeros((NHEADS, R), np.float32)
    for c in range(4):
        esel[2 * c, 128 * c:128 * c + 64] = 1.0
        esel[2 * c + 1, 128 * c + 64:128 * (c + 1)] = 1.0
    in_maps = []
    for d in range(NCORES):
        sl = slice(d * R, (d + 1) * R)
        in_maps.append({
            "xi": xi_full, "xb": xb_full,
            "adjT": np.ascontiguousarray(
                adj[sl, :].T.reshape(JT, 128, R).transpose(1, 0, 2)
                .reshape(128, JT * R)).astype(ml_dtypes.bfloat16),
            "xoT": np.ascontiguousarray(x[sl].T).astype(ml_dtypes.bfloat16),
            "ioT": np.ascontiguousarray(ie[sl].T).astype(ml_dtypes.bfloat16),
            "wqa": wqa, "qsel": qsel,
            "wia": wia, "wb": wb, "w4": w4,
            "aout": np.asarray(inputs["a_out"], np.float32),
            "esel": esel,
        })
    return in_maps


def kernel(x, adj, intent_embeds, W_sp, a_sp, W_int, a_int, W_out, a_out):
    nc = _get_nc()
    in_maps = _make_in_maps(dict(
        x=x, adj=adj, intent_embeds=intent_embeds, W_sp=W_sp, a_sp=a_sp,
        W_int=W_int, a_int=a_int, W_out=W_out, a_out=a_out))
    res = run_bass_kernel_spmd(nc, in_maps, list(range(NCORES)))
    return np.concatenate([res.results[d]["out"] for d in range(NCORES)], axis=0)
